# revision 10
# baseline (speedup 1.0000x reference)
"""Galerkin-attention encoder block on 8 TRN2 NeuronCores.

Sharding: tokens (N=8192 -> 1024/core). The only cross-core dependency is
the Galerkin contraction scores[b,h] = sum_n k[n] (x) v[n] / N, reduced with
four per-batch 512KB AllReduces that overlap local compute.

All device compute runs in "transposed space" (features on partitions,
tokens on the free axis) against a host-side pre-transposed bf16 x^T, so
the kernel needs no on-device transposes anywhere:
  qT = Wq^T x^T, attnT = scores^T qT, x1T = xT + attnT,
  hT = silu(W1^T x1T), outT = x1T + W2^T hT.
k and v stay in [token, feature] layout (their LayerNorm reduces along the
free axis and the scores matmul contracts over tokens = partitions).
"""

import numpy as np
import ml_dtypes

B, N, D = 4, 8192, 1024
H, DK = 8, 128
FF = 4096
EPS = 1e-5
N_CORES = 8
NT = N // N_CORES          # tokens per core
KC = D // 128              # feature chunks of 128
FC = FF // 128
SUP = 512                  # tokens per super-tile in phases B1/B2
NSUP = NT // SUP
SUB = 128                  # tokens per sub-tile in phase A
NSUB = SUP // SUB

_GRAPH_CACHE = {}


def _build(flags, phases=3):
    import concourse.bass as bass
    import concourse.tile as tile
    from concourse import bacc, mybir
    from contextlib import ExitStack

    has_bk, has_bv, has_b2, has_affine = flags
    f32 = mybir.dt.float32
    bf16 = mybir.dt.bfloat16

    nc = bacc.Bacc("TRN2", target_bir_lowering=False, debug=False,
                   num_devices=N_CORES)

    xTb_d = nc.dram_tensor("xTb", [B, D, NT], bf16, kind="ExternalInput")
    delta_d = nc.dram_tensor("delta", [NT], f32, kind="ExternalInput")
    wq_d = nc.dram_tensor("Wq", [D, D], bf16, kind="ExternalInput")
    wk_d = nc.dram_tensor("Wk", [D, D], bf16, kind="ExternalInput")
    wv_d = nc.dram_tensor("Wv", [D, D], bf16, kind="ExternalInput")
    w1_d = nc.dram_tensor("W1", [D, FF], bf16, kind="ExternalInput")
    w2_d = nc.dram_tensor("W2", [FF, D], bf16, kind="ExternalInput")
    bq_d = nc.dram_tensor("bq", [D], f32, kind="ExternalInput")
    b1_d = nc.dram_tensor("b1", [FF], f32, kind="ExternalInput")
    bk_d = nc.dram_tensor("bk", [D], f32, kind="ExternalInput") if has_bk else None
    bv_d = nc.dram_tensor("bv", [D], f32, kind="ExternalInput") if has_bv else None
    b2_d = nc.dram_tensor("b2", [D], f32, kind="ExternalInput") if has_b2 else None
    gamma_d = nc.dram_tensor("gamma", [D], f32, kind="ExternalInput") if has_affine else None
    beta_d = nc.dram_tensor("beta", [D], f32, kind="ExternalInput") if has_affine else None
    out_d = nc.dram_tensor("outT", [B, D, NT], f32, kind="ExternalOutput")

    sub_ = mybir.AluOpType.subtract
    mult = mybir.AluOpType.mult
    ACT = mybir.ActivationFunctionType

    with tile.TileContext(nc) as tc, ExitStack() as ctx:
        singles = ctx.enter_context(tc.tile_pool(name="singles", bufs=1))
        dram = ctx.enter_context(tc.tile_pool(name="dram", bufs=1, space="DRAM"))

        eps_t = singles.tile([128, 1], f32)
        nc.vector.memset(eps_t, EPS)
        delta_sb = singles.tile([128, NT // 128], f32)
        nc.sync.dma_start(out=delta_sb[:], in_=delta_d.ap().rearrange("(g p) -> p g", p=128))
        scores_bf = singles.tile([128, B, H, DK], bf16)

        cc_in = dram.tile([B, 128, H * DK], f32)
        cc_out = [dram.tile([128, H * DK], f32, addr_space="Shared",
                            name=f"cc_out{b}") for b in range(B)]
        h_dram = dram.tile([B, NSUP, 128, FC, SUP], bf16)
        x1_dram = dram.tile([B, NSUP, 128, KC, SUP], bf16)

        # Weights that live through phases A+B1
        w_ab1_cm = tc.tile_pool(name="w_ab1", bufs=1)
        w_ab1 = w_ab1_cm.__enter__()
        wq_sb = w_ab1.tile([128, KC, D], bf16)
        nc.sync.dma_start(out=wq_sb[:], in_=wq_d.ap().rearrange("(kc p) f -> p kc f", p=128))
        w1_sb = w_ab1.tile([128, KC, FF], bf16)
        nc.sync.dma_start(out=w1_sb[:], in_=w1_d.ap().rearrange("(kc p) f -> p kc f", p=128))
        bq_sb = w_ab1.tile([128, KC], f32)
        nc.sync.dma_start(out=bq_sb[:], in_=bq_d.ap().rearrange("(m p) -> p m", p=128))
        b1_sb = w_ab1.tile([128, FC], f32)
        nc.sync.dma_start(out=b1_sb[:], in_=b1_d.ap().rearrange("(m p) -> p m", p=128))
        if has_affine:
            gamma_sb = w_ab1.tile([128, D], f32)
            nc.sync.dma_start(out=gamma_sb[:], in_=gamma_d.ap().to_broadcast([128, D]))
            beta_sb = w_ab1.tile([128, D], f32)
            nc.sync.dma_start(out=beta_sb[:], in_=beta_d.ap().to_broadcast([128, D]))
        if has_bk:
            bk_sb = w_ab1.tile([128, D], f32)
            nc.sync.dma_start(out=bk_sb[:], in_=bk_d.ap().to_broadcast([128, D]))
        if has_bv:
            bv_sb = w_ab1.tile([128, D], f32)
            nc.sync.dma_start(out=bv_sb[:], in_=bv_d.ap().to_broadcast([128, D]))

        xTb_r = [xTb_d.ap()[b].rearrange("(kc p) t -> p kc t", p=128) for b in range(B)]

        # ---------------- Phase A: k, v, LN, partial scores, AllReduce ----
        with (
            tc.tile_pool(name="wa", bufs=1) as wa,
            tc.tile_pool(name="a_x", bufs=2) as a_x,
            tc.tile_pool(name="a_kvf", bufs=4) as a_kvf,
            tc.tile_pool(name="a_ln", bufs=4) as a_ln,
            tc.tile_pool(name="a_sc", bufs=2) as a_sc,
            tc.tile_pool(name="a_kvps", bufs=3, space="PSUM") as a_kvps,
            tc.tile_pool(name="a_sps", bufs=1, space="PSUM") as a_sps,
        ):
            wk_sb = wa.tile([128, KC, D], bf16)
            nc.sync.dma_start(out=wk_sb[:], in_=wk_d.ap().rearrange("(kc p) f -> p kc f", p=128))
            wv_sb = wa.tile([128, KC, D], bf16)
            nc.sync.dma_start(out=wv_sb[:], in_=wv_d.ap().rearrange("(kc p) f -> p kc f", p=128))

            for b in range(B):
                scores_ps = a_sps.tile([128, H, DK], f32, tag="scores")
                # Four per-head accumulation groups share each PSUM bank, and
                # a start=True matmul clears its whole bank — zero once and
                # accumulate with start=False instead.
                nc.vector.memset(scores_ps, 0.0)
                for s in range(NSUP):
                    xtb = a_x.tile([128, KC, SUP], bf16, tag="xtb")
                    nc.sync.dma_start(out=xtb[:], in_=xTb_r[b][:, :, s * SUP:(s + 1) * SUP])
                    for sb in range(NSUB):
                        gsub = s * NSUB + sb
                        tsl = bass.ts(sb, SUB)

                        def proj(w_sb, bias_sb, tag):
                            ps = a_kvps.tile([128, D], f32, tag="kv")
                            for oc in range(2):
                                for kc in range(KC):
                                    nc.tensor.matmul(
                                        ps[:, oc * 512:(oc + 1) * 512],
                                        lhsT=xtb[:, kc, tsl],
                                        rhs=w_sb[:, kc, oc * 512:(oc + 1) * 512],
                                        start=(kc == 0), stop=(kc == KC - 1))
                            if bias_sb is not None:
                                nc.vector.tensor_add(ps[:], ps[:], bias_sb[:])
                            return ps

                        k_ps = proj(wk_sb, bk_sb if has_bk else None, "k")
                        v_ps = proj(wv_sb, bv_sb if has_bv else None, "v")

                        def layernorm(ps, with_delta, tag):
                            stats = a_ln.tile([128, H, 6], f32, tag="stats", name=f"stats{tag}")
                            mv = a_ln.tile([128, H, 2], f32, tag="mv", name=f"mv{tag}")
                            for h in range(H):
                                nc.vector.bn_stats(out=stats[:, h, :], in_=ps[:, h * DK:(h + 1) * DK])
                                nc.vector.bn_aggr(out=mv[:, h, :], in_=stats[:, h, :])
                            rstd = a_ln.tile([128, H], f32, tag="rstd", name=f"rstd{tag}")
                            nc.scalar.activation(out=rstd[:], in_=mv[:, :, 1], func=ACT.Sqrt, bias=eps_t[:])
                            nc.vector.reciprocal(out=rstd[:], in_=rstd[:])
                            out_t = a_kvf.tile([128, D], bf16, tag="kvf", name=f"kvf{tag}")
                            if not has_affine and with_delta:
                                nc.vector.tensor_scalar_mul(
                                    out=rstd[:], in0=rstd[:],
                                    scalar1=delta_sb[:, gsub:gsub + 1])
                            for h in range(H):
                                hs = slice(h * DK, (h + 1) * DK)
                                nc.vector.tensor_scalar(
                                    out=out_t[:, hs], in0=ps[:, hs],
                                    scalar1=mv[:, h, 0:1], scalar2=rstd[:, h:h + 1],
                                    op0=sub_, op1=mult)
                            if has_affine:
                                nc.vector.tensor_mul(out_t[:], out_t[:], gamma_sb[:])
                                nc.vector.tensor_add(out_t[:], out_t[:], beta_sb[:])
                                if with_delta:
                                    nc.vector.tensor_scalar_mul(
                                        out=out_t[:], in0=out_t[:],
                                        scalar1=delta_sb[:, gsub:gsub + 1])
                            return out_t

                        kf = layernorm(k_ps, True, "k")
                        vf = layernorm(v_ps, False, "v")

                        for h in range(H):
                            hs = slice(h * DK, (h + 1) * DK)
                            nc.tensor.matmul(
                                scores_ps[:, h, :], lhsT=kf[:, hs], rhs=vf[:, hs],
                                start=False,
                                stop=(s == NSUP - 1 and sb == NSUB - 1),
                                skip_group_check=True)

                sc_sb = a_sc.tile([128, H * DK], f32, tag="scsb")
                nc.scalar.copy(out=sc_sb[:], in_=scores_ps[:, :, :])
                nc.sync.dma_start(out=cc_in[b], in_=sc_sb[:])
                nc.gpsimd.collective_compute(
                    "AllReduce", mybir.AluOpType.add,
                    replica_groups=[list(range(N_CORES))],
                    ins=[cc_in[b].opt()], outs=[cc_out[b].opt()])

        if phases < 1:
            # Debug: dump reduced scores straight to outT and stop.
            with tc.tile_pool(name="dbg", bufs=1) as dbg:
                sc_dbg = dbg.tile([128, B, H * DK], f32)
                for b in range(B):
                    nc.sync.dma_start(out=sc_dbg[:, b, :], in_=cc_out[b])
                    nc.sync.dma_start(out=out_d.ap()[b][0:128, :],
                                      in_=sc_dbg[:, b, 0:NT])
            w_ab1_cm.__exit__(None, None, None)
            nc.finalize()
            return nc

        # ---------------- Phase B1: qT, attn, x1T, first FFN matmul -------
        with (
            tc.tile_pool(name="b1_x", bufs=2) as b1_x,
            tc.tile_pool(name="b1_q", bufs=2) as b1_q,
            tc.tile_pool(name="b1_x1", bufs=2) as b1_x1,
            tc.tile_pool(name="b1_h", bufs=2) as b1_h,
            tc.tile_pool(name="b1_sc", bufs=1) as b1_sc,
            tc.tile_pool(name="b1_qps", bufs=2, space="PSUM") as b1_qps,
            tc.tile_pool(name="b1_aps", bufs=2, space="PSUM") as b1_aps,
            tc.tile_pool(name="b1_hps", bufs=3, space="PSUM") as b1_hps,
        ):
            sc_f = b1_sc.tile([128, B, H * DK], f32)
            for b in range(B):
                nc.sync.dma_start(out=sc_f[:, b, :], in_=cc_out[b])
                nc.vector.tensor_copy(out=scores_bf[:, b, :, :], in_=sc_f[:, b, :])

            for b in range(B):
                for s in range(NSUP):
                    xtb = b1_x.tile([128, KC, SUP], bf16, tag="xtb1")
                    nc.sync.dma_start(out=xtb[:], in_=xTb_r[b][:, :, s * SUP:(s + 1) * SUP])

                    qt = b1_q.tile([128, H, SUP], bf16, tag="qt")
                    for m in range(KC):
                        q_ps = b1_qps.tile([128, SUP], f32, tag="qps")
                        for kc in range(KC):
                            nc.tensor.matmul(
                                q_ps[:], lhsT=wq_sb[:, kc, m * 128:(m + 1) * 128],
                                rhs=xtb[:, kc, :],
                                start=(kc == 0), stop=(kc == KC - 1))
                        nc.scalar.activation(out=qt[:, m, :], in_=q_ps[:],
                                             func=ACT.Identity, bias=bq_sb[:, m:m + 1])

                    x1 = b1_x1.tile([128, KC, SUP], bf16, tag="x1")
                    for h in range(H):
                        a_ps = b1_aps.tile([128, SUP], f32, tag="aps")
                        nc.tensor.matmul(a_ps[:], lhsT=scores_bf[:, b, h, :],
                                         rhs=qt[:, h, :], start=True, stop=True)
                        nc.vector.tensor_add(x1[:, h, :], a_ps[:], xtb[:, h, :])
                    nc.gpsimd.dma_start(out=x1_dram[b, s], in_=x1[:])

                    for g in range(4):
                        hsb = b1_h.tile([128, 8, SUP], bf16, tag="hsb")
                        for mm in range(8):
                            m = g * 8 + mm
                            h_ps = b1_hps.tile([128, SUP], f32, tag="hps")
                            for kc in range(KC):
                                nc.tensor.matmul(
                                    h_ps[:], lhsT=w1_sb[:, kc, m * 128:(m + 1) * 128],
                                    rhs=x1[:, kc, :],
                                    start=(kc == 0), stop=(kc == KC - 1))
                            nc.scalar.activation(out=hsb[:, mm, :], in_=h_ps[:],
                                                 func=ACT.Silu, bias=b1_sb[:, m:m + 1])
                        nc.gpsimd.dma_start(out=h_dram[b, s, :, g * 8:(g + 1) * 8, :], in_=hsb[:])

        w_ab1_cm.__exit__(None, None, None)

        if phases < 2:
            # Debug: dump x1 straight to outT and stop.
            with tc.tile_pool(name="dbg2", bufs=2) as dbg2:
                for b in range(B):
                    for s in range(NSUP):
                        x1d = dbg2.tile([128, KC, SUP], bf16, tag="x1d")
                        nc.sync.dma_start(out=x1d[:], in_=x1_dram[b, s])
                        x1f = dbg2.tile([128, KC, SUP], f32, tag="x1f")
                        nc.vector.tensor_copy(out=x1f[:], in_=x1d[:])
                        out_r_b = out_d.ap()[b].rearrange("(m p) t -> p m t", p=128)
                        nc.sync.dma_start(out=out_r_b[:, :, s * SUP:(s + 1) * SUP],
                                          in_=x1f[:])
            nc.finalize()
            return nc

        # ---------------- Phase B2: yT + residual -------------------------
        with (
            tc.tile_pool(name="w_b2", bufs=1) as w_b2,
            tc.tile_pool(name="b2_h", bufs=2) as b2_h,
            tc.tile_pool(name="b2_x1", bufs=2) as b2_x1,
            tc.tile_pool(name="b2_o", bufs=2) as b2_o,
            tc.tile_pool(name="b2_yps", bufs=4, space="PSUM") as b2_yps,
        ):
            w2_sb = w_b2.tile([128, FC, D], bf16)
            nc.sync.dma_start(out=w2_sb[:], in_=w2_d.ap().rearrange("(kc p) f -> p kc f", p=128))
            if has_b2:
                b2_sb = w_b2.tile([128, KC], f32)
                nc.sync.dma_start(out=b2_sb[:], in_=b2_d.ap().rearrange("(m p) -> p m", p=128))

            out_r = [out_d.ap()[b].rearrange("(m p) t -> p m t", p=128) for b in range(B)]
            for b in range(B):
                for s in range(NSUP):
                    ht = b2_h.tile([128, FC, SUP], bf16, tag="ht")
                    nc.sync.dma_start(out=ht[:], in_=h_dram[b, s])
                    x1r = b2_x1.tile([128, KC, SUP], bf16, tag="x1r")
                    nc.sync.dma_start(out=x1r[:], in_=x1_dram[b, s])
                    ot = b2_o.tile([128, KC, SUP], f32, tag="ot")
                    for m in range(KC):
                        y_ps = b2_yps.tile([128, SUP], f32, tag="yps")
                        for kc in range(FC):
                            nc.tensor.matmul(
                                y_ps[:], lhsT=w2_sb[:, kc, m * 128:(m + 1) * 128],
                                rhs=ht[:, kc, :],
                                start=(kc == 0), stop=(kc == FC - 1))
                        if has_b2:
                            nc.vector.tensor_scalar_add(out=y_ps[:], in0=y_ps[:],
                                                        scalar1=b2_sb[:, m:m + 1])
                        nc.vector.tensor_add(ot[:, m, :], y_ps[:], x1r[:, m, :])
                    nc.gpsimd.dma_start(out=out_r[b][:, :, s * SUP:(s + 1) * SUP], in_=ot[:])

    nc.finalize()
    return nc


def _get_graph(flags):
    if flags not in _GRAPH_CACHE:
        _GRAPH_CACHE[flags] = _build(flags)
    return _GRAPH_CACHE[flags]


def kernel(x, delta_x, Wq, bq, Wk, bk, Wv, bv, gamma_k, beta_k, W1, b1, W2, b2,
           _trace=False):
    from concourse.bass_utils import run_bass_kernel_spmd

    bf = ml_dtypes.bfloat16
    x = np.asarray(x, np.float32)
    delta_x = np.asarray(delta_x, np.float32)
    Wq, Wk, Wv = (np.asarray(w, np.float32) for w in (Wq, Wk, Wv))
    W1, W2 = np.asarray(W1, np.float32), np.asarray(W2, np.float32)
    bq, bk, bv = (np.asarray(v, np.float32) for v in (bq, bk, bv))
    b1, b2 = np.asarray(b1, np.float32), np.asarray(b2, np.float32)
    gamma_k = np.asarray(gamma_k, np.float32)
    beta_k = np.asarray(beta_k, np.float32)

    has_bk = bool(np.any(bk))
    has_bv = bool(np.any(bv))
    has_b2 = bool(np.any(b2))
    has_affine = not (np.all(gamma_k == 1.0) and np.all(beta_k == 0.0))
    flags = (has_bk, has_bv, has_b2, has_affine)
    nc = _get_graph(flags)

    wq_b, wk_b, wv_b = Wq.astype(bf), Wk.astype(bf), Wv.astype(bf)
    w1_b, w2_b = W1.astype(bf), W2.astype(bf)
    delta_pre = (delta_x / np.float32(N)).astype(np.float32)

    in_maps = []
    for c in range(N_CORES):
        t0 = c * NT
        xT = np.ascontiguousarray(x[:, t0:t0 + NT, :].transpose(0, 2, 1)).astype(bf)
        m = {"xTb": xT, "delta": np.ascontiguousarray(delta_pre[t0:t0 + NT]),
             "Wq": wq_b, "Wk": wk_b, "Wv": wv_b, "W1": w1_b, "W2": w2_b,
             "bq": bq, "b1": b1}
        if has_bk:
            m["bk"] = bk
        if has_bv:
            m["bv"] = bv
        if has_b2:
            m["b2"] = b2
        if has_affine:
            m["gamma"] = gamma_k.reshape(D).copy()
            m["beta"] = beta_k.reshape(D).copy()
        in_maps.append(m)

    res = run_bass_kernel_spmd(nc, in_maps, core_ids=list(range(N_CORES)),
                               trace=_trace)

    out = np.empty((B, N, D), np.float32)
    for c in range(N_CORES):
        t0 = c * NT
        out[:, t0:t0 + NT, :] = res.results[c]["outT"].transpose(0, 2, 1)
    if _trace:
        return out, res
    return out
